# revision 27
# baseline (speedup 1.0000x reference)
"""GAT-style GNN message passing on 8 TRN2 NeuronCores — no collectives.

Math: with LEAK=1 the leaky-relu is identity, so
  e[i,j,h] = e_src[i,h] + e_dst[j,h]
and softmax over j cancels e_src (and any row max) exactly:
  attn[i,j,h] = adj[i,j]*exp(e_dst[j,h]) / sum_j adj[i,j]*exp(e_dst[j,h])
  out[i,(h,f)] = (adj @ (z*h))[i,(h,f)] / (adj @ z)[i,h],  z = exp(e_dst)
then elu + log_softmax per row. log_softmax is shift invariant, so
elu(x) is computed as relu(x) + exp(min(x,0)) (drops the uniform -1).

Sharding: ROW-shard adj/out only; REPLICATE the h computation. The
previous all-gather design lost ~80us to the collective stack (38.7us
entry barrier + 2x ~14us RDH AllGathers for 74KB payloads). Instead
every core loads the full x (fp8, 4MB) and computes h/z/G for all 4096
nodes locally (~14us extra PE time), then aggregates its own
[512, 4096] adjacency slab. Zero cross-core traffic.

Precision: all matmul inputs are fp8 e4m3. adj entries (0/1) are exact
in fp8. x/W quantization errors average out over the 1024-deep (h) and
~2048-deep (aggregation) contractions; measured end-to-end rel err is
~1e-3 vs the 2e-2 gate. W columns are pre-scaled by 8 (and the fused
a_dst columns by 32) to avoid fp8-subnormal truncation; the scales are
divided back out in the on-chip postprocessing (exp has a scale arg,
the g-multiply uses scalar_tensor_tensor).

Per-core device program (R = N/8 = 512 rows, P = 128):
  inputs:  xt [128, 8*8*512] fp8   xt[p, c*4096+k*512+n] = x[c*512+n, k*128+p]
           wt [128, 8*80]    fp8   wt[p, k*80+e] = w_ext[k*128+p, e] (e<72)
           at [128, 32*512]  fp8   at[p, j*512+r] = adj[core*512+r, j*128+p]
  output:  out_p [128, 4*64] f32   out_p[p, q*64+f] = out[core*512+q*128+p, f]

Pipeline: h-matmul (64 MMs, fp8) -> PE transposes of the 32 [72,128]
hT chunks into PSUM (bf16) -> batched exp/multiply builds G=[h*z | z]
in fp8 -> 32 accumulating aggregation MMs against the adj slab -> small
transposed postprocess (reciprocal, elu, log-softmax) -> one output DMA.
"""

import sys

import numpy as np

if "/opt/trn_rl_repo" not in sys.path:
    sys.path.insert(0, "/opt/trn_rl_repo")

import ml_dtypes  # noqa: E402

import concourse.bass as bass  # noqa: E402
import concourse.tile as tile  # noqa: E402
from concourse import bacc, mybir  # noqa: E402
from concourse.bass_utils import run_bass_kernel_spmd  # noqa: E402
from concourse.masks import make_identity  # noqa: E402

N_CORES = 8
N_NODES = 4096
H = 8
F = 8
HF = H * F  # 64
EXT = HF + H  # 72: [h | e_dst]
EXTP = 80  # padded slot width (fp8 bytes) so DoubleRow strides are %16
K_IN = 1024
P = 128
KC = K_IN // P  # 8 k-chunks
CC = N_NODES // 512  # 8 column chunks for the h matmul
NC = N_NODES // P  # 32 j-chunks for the aggregation
R = N_NODES // N_CORES  # 512 rows per core
RC = R // P  # 4 output chunks per core

S_W = 8.0  # host pre-scale on W columns (fp8 subnormal avoidance)
S_D = 32.0  # host pre-scale on the fused a_dst columns

USE_DOUBLE_ROW = True  # fp8 DoubleRow perf mode on the big matmuls
N_WARMUP_MM = 8  # wide dummy matmuls (~3.4us cold) to trip the PE HAM gate

FP32 = mybir.dt.float32
BF16 = mybir.dt.bfloat16
FP8 = mybir.dt.float8e4
NP_FP8 = ml_dtypes.float8_e4m3
AFT = mybir.ActivationFunctionType
ALU = mybir.AluOpType


def _bcast_f(ap_pch):
    """[..., H] AP -> [..., H, F] AP broadcasting each head value over F."""
    return bass.AP(
        tensor=ap_pch.tensor,
        offset=ap_pch.offset,
        ap=list(ap_pch.ap) + [[0, F]],
    )


def build_bass() -> bass.Bass:
    nc = bacc.Bacc(num_devices=N_CORES)

    xt = nc.declare_dram_parameter("xt", [P, CC * KC * 512], FP8, isOutput=False)
    wt = nc.declare_dram_parameter("wt", [P, KC * EXTP], FP8, isOutput=False)
    at = nc.declare_dram_parameter("at", [P, NC * 512], FP8, isOutput=False)
    out = nc.declare_dram_parameter("out", [P, RC * HF], FP32, isOutput=True)

    with tile.TileContext(nc) as tc:
        with (
            tc.tile_pool(name="singles", bufs=1) as singles,
            tc.tile_pool(name="hps", bufs=2, space="PSUM") as hps,
            tc.tile_pool(name="tps", bufs=1, space="PSUM") as tps,
            tc.tile_pool(name="aps", bufs=1, space="PSUM") as aps,
            tc.tile_pool(name="ops", bufs=1, space="PSUM") as ops,
            tc.tile_pool(name="work", bufs=2) as work,
        ):
            ident_bf = singles.tile([P, P], BF16)
            make_identity(nc, ident_bf)

            # --- loads (p-major, one contiguous run per partition) ---
            # Sync HWDGE is a FIFO: wt (tiny) drains first, then a small
            # first xt chunk so the first h-matmul starts ~1.5us earlier,
            # then the bulk in 1MB chunks (past the DMA-size knee).
            wt_sb = singles.tile([P, KC, EXTP], FP8)
            nc.sync.dma_start(
                out=wt_sb, in_=wt[:].rearrange("p (k e) -> p k e", k=KC)
            )
            xt_sb = singles.tile([P, CC, KC, 512], FP8)
            xt_view = xt[:].rearrange("p (c k n) -> p c k n", c=CC, k=KC)
            for c in range(0, CC, 2):
                nc.sync.dma_start(out=xt_sb[:, c : c + 2], in_=xt_view[:, c : c + 2])
            at_sb = singles.tile([P, NC, 512], FP8)
            at_view = at[:].rearrange("p (j r) -> p j r", j=NC)
            for lo, hi in ((0, 24), (24, 32)):  # small last piece: less
                # aggregation work left after the final byte lands
                nc.sync.dma_start(out=at_sb[:, lo:hi], in_=at_view[:, lo:hi])

            # Early throwaway Exp so the compiler's ACT_TABLE_LOAD lands
            # here (under the DMA/warmup window) instead of on the
            # critical path before the first real Exp.
            tbl = work.tile([1, 1], FP32, tag="tbl")
            nc.scalar.activation(tbl, ident_bf[0:1, 0:1], AFT.Exp)

            # --- postprocess PSUM tile (bf16 transposed output chunks) ---
            o_ps = ops.tile([P, RC, P], BF16)  # 256B slots, 8/bank

            # --- PE warmup: wide matmuls on a zeroed scratch tile trip the
            # HAM activity window while the first xt DMA is in flight, so
            # the real matmuls run at 2.4GHz from the start. Scratch target
            # is the aggregation bank — its accumulation group clears the
            # whole bank on its first matmul anyway. ---
            outT_ps = aps.tile([EXT, 512], FP32)
            warm_rhs = singles.tile([P, 512], BF16)
            nc.gpsimd.memset(warm_rhs, 0.0)
            for i in range(N_WARMUP_MM):
                nc.tensor.matmul(
                    outT_ps[0:64, :],
                    lhsT=ident_bf[:, 0:64],
                    rhs=warm_rhs,
                    start=True,
                    stop=True,
                )

            # --- hT = w_ext.T @ x.T : [72, 4096] fp8 matmuls, fp32 PSUM.
            # PE transposes trail the matmuls by two chunks so they never
            # stall on the PSUM->SBUF evacuation (all evacs on DVE); the
            # z/g construction for each half is emitted mid-loop so the
            # Scalar/Vector streams reach it as soon as its data is ready.
            hT_sb = singles.tile([EXT, CC, 512], BF16)
            tr_ps = tps.tile([P, NC, P], BF16)  # 256B slots, 8/bank, 4 banks
            z_all = singles.tile([P, NC, H], BF16)
            g_ext = singles.tile([P, NC, EXTP], FP8)

            def do_transposes(c):
                for q in range(4):
                    j = c * 4 + q
                    nc.tensor.transpose(
                        tr_ps[:, j, :EXT],
                        hT_sb[:, c, q * P : (q + 1) * P],
                        ident_bf[:EXT, :EXT],
                    )

            def do_zg(s):
                # one quarter: 8 j-chunks (two transposed hT chunks)
                sl = slice(8 * s, 8 * (s + 1))
                nc.scalar.activation(
                    z_all[:, sl, :], tr_ps[:, sl, HF:EXT], AFT.Exp, scale=1.0 / S_D
                )
                nc.vector.scalar_tensor_tensor(
                    out=g_ext[:, sl, 0:HF].rearrange("p c (h f) -> p c h f", h=H),
                    in0=tr_ps[:, sl, 0:HF].rearrange("p c (h f) -> p c h f", h=H),
                    scalar=1.0 / S_W,
                    in1=_bcast_f(z_all[:, sl, :]),
                    op0=ALU.mult,
                    op1=ALU.mult,
                )
                nc.vector.tensor_copy(g_ext[:, sl, HF:EXT], z_all[:, sl, :])

            for c in range(CC):
                hT_ps = hps.tile([EXT, 512], FP32, tag="hps", name=f"hT{c}")
                if USE_DOUBLE_ROW:
                    for t in range(KC // 2):
                        nc.tensor.matmul(
                            hT_ps,
                            lhsT=wt_sb[:, 2 * t : 2 * t + 2, :EXT],
                            rhs=xt_sb[:, c, 2 * t : 2 * t + 2, :],
                            start=(t == 0),
                            stop=(t == KC // 2 - 1),
                            perf_mode=mybir.MatmulPerfMode.DoubleRow,
                        )
                else:
                    for k in range(KC):
                        nc.tensor.matmul(
                            hT_ps,
                            lhsT=wt_sb[:, k, :EXT],
                            rhs=xt_sb[:, c, k, :],
                            start=(k == 0),
                            stop=(k == KC - 1),
                        )
                # evacuate to bf16, split across Vector+Scalar so the
                # dependent transposes wait half as long and neither
                # engine queue becomes the serial gate
                nc.vector.tensor_copy(hT_sb[:, c, 0:256], hT_ps[:, 0:256])
                nc.scalar.activation(
                    hT_sb[:, c, 256:512], hT_ps[:, 256:512], AFT.Copy
                )
                if c >= 2:
                    do_transposes(c - 2)
                if c >= 3 and c % 2 == 1:
                    do_zg((c - 3) // 2)  # its transposed chunks just landed
            do_transposes(CC - 2)
            do_transposes(CC - 1)
            do_zg(3)

            # --- aggregation: outT[72, 512] += G_j.T @ adjT_j over 32 chunks ---
            if USE_DOUBLE_ROW:
                for t in range(NC // 2):
                    nc.tensor.matmul(
                        outT_ps,
                        lhsT=g_ext[:, 2 * t : 2 * t + 2, 0:EXT],
                        rhs=at_sb[:, 2 * t : 2 * t + 2, :],
                        start=(t == 0),
                        stop=(t == NC // 2 - 1),
                        perf_mode=mybir.MatmulPerfMode.DoubleRow,
                    )
            else:
                for j in range(NC):
                    nc.tensor.matmul(
                        outT_ps,
                        lhsT=g_ext[:, j, 0:EXT],
                        rhs=at_sb[:, j, :],
                        start=(j == 0),
                        stop=(j == NC - 1),
                    )
            outT_sb = singles.tile([EXT, 512], BF16)
            nc.vector.tensor_copy(outT_sb, outT_ps)

            # --- postprocess: x = num/den, elu+1, log_softmax ---
            # Two q-halves pipelined across the Vector/Scalar engines:
            # half B's Vector ops run under half A's Scalar ops and the
            # first store issues while half B is still in flight.
            out_sb = singles.tile([P, RC, HF], FP32)
            out_view = out[:].rearrange("p (q f) -> p q f", q=RC)
            for hh in range(2):
                sl = slice(2 * hh, 2 * hh + 2)
                for q in range(2 * hh, 2 * hh + 2):
                    nc.tensor.transpose(
                        o_ps[:, q, :EXT],
                        outT_sb[:, q * P : (q + 1) * P],
                        ident_bf[:EXT, :EXT],
                    )
                rd = work.tile([P, 2, H], FP32, tag="rd", name=f"rd{hh}")
                nc.vector.reciprocal(rd, o_ps[:, sl, HF:EXT])
                xo = work.tile([P, 2, HF], FP32, tag="xo", name=f"xo{hh}")
                nc.vector.tensor_mul(
                    xo[:].rearrange("p q (h f) -> p q h f", h=H),
                    o_ps[:, sl, 0:HF].rearrange("p q (h f) -> p q h f", h=H),
                    _bcast_f(rd[:]),
                )
                # y = relu(x) + exp(min(x,0))  (elu + 1; log_softmax shift-safe)
                mo = work.tile([P, 2, HF], FP32, tag="mo", name=f"mo{hh}")
                nc.vector.tensor_scalar_min(mo, xo, 0.0)
                eo = work.tile([P, 2, HF], FP32, tag="eo", name=f"eo{hh}")
                nc.scalar.activation(eo, mo, AFT.Exp)
                yo = work.tile([P, 2, HF], FP32, tag="yo", name=f"yo{hh}")
                nc.vector.scalar_tensor_tensor(
                    out=yo, in0=xo, scalar=0.0, in1=eo, op0=ALU.max, op1=ALU.add
                )
                ex = work.tile([P, 2, HF], FP32, tag="ex", name=f"ex{hh}")
                nc.scalar.activation(ex, yo, AFT.Exp)
                sm = work.tile([P, 2], FP32, tag="sm", name=f"sm{hh}")
                nc.vector.reduce_sum(sm, ex, axis=mybir.AxisListType.X)
                ls = work.tile([P, 2], FP32, tag="ls", name=f"ls{hh}")
                nc.scalar.activation(ls, sm, AFT.Ln)
                ls_b = bass.AP(
                    tensor=ls[:].tensor,
                    offset=ls[:].offset,
                    ap=list(ls[:].ap) + [[0, HF]],
                )
                nc.vector.tensor_sub(out_sb[:, sl], yo, ls_b)
                nc.sync.dma_start(out=out_view[:, sl], in_=out_sb[:, sl])

    # Force all ACT activations (Exp + Ln) onto the one table set containing
    # both, so only ONE ACT_TABLE_LOAD is emitted (early, hidden under DMA)
    # instead of a ~1.3us reload at every Exp<->Ln switch.
    orig_gat = bacc.get_activation_tables

    def _one_set(arch):
        return {
            k: (v if k == "natural_log_exp_and_others" else set())
            for k, v in orig_gat(arch).items()
        }

    bacc.get_activation_tables = _one_set
    try:
        nc.finalize()
    finally:
        bacc.get_activation_tables = orig_gat
    return nc


def _host_prep(x, adj, W, a_dst):
    """Build per-core input maps (xt/wt replicated, at row-sharded)."""
    Wd = np.einsum(
        "khf,hf->kh", W.reshape(K_IN, H, F), a_dst, dtype=np.float32
    ).astype(np.float32)
    w_ext = np.concatenate([W * S_W, Wd * S_D], axis=1)  # [1024, 72]
    wt_np = np.zeros((P, KC, EXTP), dtype=NP_FP8)
    wt_np[:, :, :EXT] = (
        w_ext.reshape(KC, P, EXT).transpose(1, 0, 2).astype(NP_FP8)
    )
    wt_np = wt_np.reshape(P, KC * EXTP)

    x8 = x.astype(NP_FP8)  # [4096, 1024]
    # xt[p, c, k, n] = x8[c*512+n, k*128+p]
    xt_np = np.ascontiguousarray(
        x8.reshape(CC, 512, KC, P).transpose(3, 0, 2, 1)
    ).reshape(P, CC * KC * 512)

    adj8 = (adj > 0).astype(NP_FP8)  # [4096, 4096]
    in_maps = []
    for c in range(N_CORES):
        rows = slice(c * R, (c + 1) * R)
        # at[p, j, r] = adj8[c*R+r, j*128+p]
        at_np = np.ascontiguousarray(
            adj8[rows].reshape(R, NC, P).transpose(2, 1, 0)
        ).reshape(P, NC * R)
        in_maps.append({"xt": xt_np, "wt": wt_np, "at": at_np})
    return in_maps


_BUILT = {}


def run(x, adj, W, a_dst, trace=False):
    if "nc" not in _BUILT:
        _BUILT["nc"] = build_bass()
    nc = _BUILT["nc"]
    in_maps = _host_prep(x, adj, W, a_dst)
    res = run_bass_kernel_spmd(nc, in_maps, list(range(N_CORES)), trace=trace)
    blocks = []
    for c in range(N_CORES):
        o = res.results[c]["out"]  # [P, RC*HF] p-major
        blocks.append(o.reshape(P, RC, HF).transpose(1, 0, 2).reshape(R, HF))
    return np.concatenate(blocks, axis=0).astype(np.float32), res


def kernel(x, adj, W, a_src, a_dst):
    x = np.asarray(x, dtype=np.float32)
    adj = np.asarray(adj)
    W = np.asarray(W, dtype=np.float32)
    a_dst = np.asarray(a_dst, dtype=np.float32)
    out, _ = run(x, adj, W, a_dst, trace=False)
    return out
